# revision 7
# baseline (speedup 1.0000x reference)
"""Multi-head attention (B=2, S=2048, H=2048, NH=16) on 8 TRN2 NeuronCores.

Sharding: tensor-parallel over heads - 2 heads per core. Each core computes
q/k/v projections for its heads, per-head attention, and a partial output
projection (its heads' columns of Wo); the host sums the 8 partials.

v2: fine-interleaved schedule.
  - Unified PSUM ring ps_mix (3 bufs x 2 banks) for scores/den/proj/oproj,
    plus a dedicated single-buf AV accumulator (2 banks) = 8 banks.
  - Attention emitted as generator groups (one (b,qh,h) group at a time);
    projection/o-proj work is drained from a filler queue every few tcx
    steps and between groups, so the PE stream never starves.
  - AV matmuls lag scores by one tcx so they never wait on the exp.
  - qt/kt bias-adds on ScalarE (Identity+bias), v bias on DVE.
  - O-proj psum->bf16 casts alternate DVE/ScalarE; output stores alternate
    sync/gpsimd DMA queues.
  - hT quarters prefetched through a 4-deep pool starting before weights.
"""

import sys

sys.path.insert(0, "/opt/trn_rl_repo")

from collections import deque
from contextlib import ExitStack

import ml_dtypes
import numpy as np

import concourse.bass as bass
import concourse.tile as tile
from concourse import bacc, mybir
from concourse.bass_utils import run_bass_kernel_spmd

B, S, H, NH = 2, 2048, 2048, 16
HD = H // NH          # 128
N_CORES = 8
HPC = NH // N_CORES   # heads per core = 2
HDC = HPC * HD        # head-dims per core = 256
T = B * S             # 4096 tokens
FC = H // 128         # 16 feature chunks
TC = S // 128         # 16 token tiles per batch
SHIFT = 4.0           # fixed exp shift (softmax-invariant, overflow guard)

BF16 = mybir.dt.bfloat16
F32 = mybir.dt.float32
EXP = mybir.ActivationFunctionType.Exp
COPY = mybir.ActivationFunctionType.Copy
IDENT = mybir.ActivationFunctionType.Identity

_CACHE = {}


def build_program(out_dtype=BF16):
    nc = bacc.Bacc(
        "TRN2", target_bir_lowering=False, debug=False, num_devices=N_CORES
    )
    # hTq: hT pre-chunked on the host into SBUF tile layout: quarter q
    # (= (b, half, qx)) holds [128, FC, 512] with 16KB contiguous per
    # partition -> one DMA descriptor per partition row.
    hTq = nc.dram_tensor("hTq", [8, 128, FC, 512], BF16, kind="ExternalInput").ap()
    wqT = nc.dram_tensor("wqT", [128, FC, HDC], BF16, kind="ExternalInput").ap()
    wkT = nc.dram_tensor("wkT", [128, FC, HDC], BF16, kind="ExternalInput").ap()
    wvT = nc.dram_tensor("wvT", [128, FC, HDC], BF16, kind="ExternalInput").ap()
    woT = nc.dram_tensor("woT", [128, HPC, H], BF16, kind="ExternalInput").ap()
    bq = nc.dram_tensor("bq", [HDC], F32, kind="ExternalInput").ap()
    bk = nc.dram_tensor("bk", [HDC], F32, kind="ExternalInput").ap()
    bv = nc.dram_tensor("bv", [1, HDC], F32, kind="ExternalInput").ap()
    out = nc.dram_tensor("out", [T, H], out_dtype, kind="ExternalOutput").ap()

    with tile.TileContext(nc) as tc:
        _kernel(tc, out, hTq, wqT, wkT, wvT, woT, bq, bk, bv)
    nc.compile()
    return nc


def _kernel(tc, out, hTq, wqT, wkT, wvT, woT, bq, bk, bv):
    nc = tc.nc
    scale = 1.0 / float(np.sqrt(HD))
    ctx = ExitStack()
    with ctx:
        singles = ctx.enter_context(tc.tile_pool(name="singles", bufs=1))
        persist = ctx.enter_context(tc.tile_pool(name="persist", bufs=1))
        ps_mix = ctx.enter_context(tc.tile_pool(name="ps_mix", bufs=3, space="PSUM"))
        ps_av = ctx.enter_context(tc.tile_pool(name="ps_av", bufs=1, space="PSUM"))
        ht_pool = ctx.enter_context(tc.tile_pool(name="ht", bufs=3))
        pt_pool = ctx.enter_context(tc.tile_pool(name="pt", bufs=9))
        pair_pool = ctx.enter_context(tc.tile_pool(name="pair", bufs=2))
        quad_pool = ctx.enter_context(tc.tile_pool(name="quad", bufs=4))
        oct_pool = ctx.enter_context(tc.tile_pool(name="oct", bufs=2))
        hex_pool = ctx.enter_context(tc.tile_pool(name="hex", bufs=1))
        den_pool = ctx.enter_context(tc.tile_pool(name="den", bufs=1))
        o_sb_pool = ctx.enter_context(tc.tile_pool(name="o_sb", bufs=4))
        of_pool = ctx.enter_context(tc.tile_pool(name="of", bufs=2))

        # ---- hT prefetch machinery (sync queue) ----
        ht_tiles = {}

        def prefetch_ht(b, half, qx, engs=None):
            # split each quarter into 4 fc-range pieces alternating between
            # the sync and scalar DMA queues, so the first fc chunks land
            # quickly and consumers start early (region-level deps)
            qidx = b * 4 + half * 2 + qx
            t = ht_pool.tile([128, FC, 512], BF16, tag="ht",
                             name=f"ht{b}{half}{qx}")
            if engs is None:
                engs = (nc.sync, nc.gpsimd)
            for g in range(4):
                eng = engs[g % len(engs)]
                src = bass.AP(
                    tensor=hTq.tensor,
                    offset=hTq.offset + (qidx * 128 * FC + 4 * g) * 512,
                    ap=[[FC * 512, 128], [512, 4], [1, 512]],
                )
                eng.dma_start(out=t[:, 4 * g : 4 * g + 4, :], in_=src)
            ht_tiles[(b, half, qx)] = t

        bq_sb = singles.tile([128, HPC], F32)
        bk_sb = singles.tile([128, HPC], F32)
        # bv broadcast across partitions once ([128, HDC]); the 2-wide group
        # dim is a free-dim stride-0 view at use time
        bv_sb = singles.tile([128, HDC], F32)
        bv2 = bass.AP(tensor=bv_sb.tensor, offset=bv_sb.offset,
                      ap=[bv_sb.ap[0], [0, 2], [1, HDC]])

        # startup choreography: round-robin units across the three DMA
        # queues in consumption order, so each unit lands just before its
        # matmuls. The very first ht00/wv pieces are 1-fc sized (0.125MB)
        # so the first v matmuls start as early as possible.
        w_sb = {}
        for name, ap in (("v", wvT), ("q", wqT), ("k", wkT)):
            w_sb[name] = singles.tile([128, FC, HDC], BF16, tag=f"w{name}",
                                      name=f"w{name}")

        def w_unit(name, f0, f1):  # fc range [f0, f1)
            src = (wvT, wqT, wkT)[("v", "q", "k").index(name)]
            return (w_sb[name][:, f0:f1, :], src[:, f0:f1, :])

        def ht_unit(qidx, f0, f1, t):
            src = bass.AP(
                tensor=hTq.tensor,
                offset=hTq.offset + (qidx * 128 * FC + f0) * 512,
                ap=[[FC * 512, 128], [512, f1 - f0], [1, 512]],
            )
            return (t[:, f0:f1, :], src)

        ht00 = ht_pool.tile([128, FC, 512], BF16, tag="ht", name="ht000")
        ht01 = ht_pool.tile([128, FC, 512], BF16, tag="ht", name="ht001")
        ht_tiles[(0, 0, 0)] = ht00
        ht_tiles[(0, 0, 1)] = ht01
        units = []
        for fc in range(2):  # fine-grained head of the stream
            units.append(ht_unit(0, fc, fc + 1, ht00))
            units.append(w_unit("v", fc, fc + 1))
        units.append(ht_unit(0, 2, 4, ht00))
        units.append(w_unit("v", 2, 4))
        for g in range(1, 4):
            units.append(ht_unit(0, 4 * g, 4 * g + 4, ht00))
            units.append(w_unit("v", 4 * g, 4 * g + 4))
        for g in range(4):
            units.append(ht_unit(1, 4 * g, 4 * g + 4, ht01))
        for g in range(4):
            units.append(w_unit("q", 4 * g, 4 * g + 4))
        for g in range(4):
            units.append(w_unit("k", 4 * g, 4 * g + 4))
        qs = (nc.sync, nc.gpsimd, nc.scalar)
        for i, (dst, src) in enumerate(units):
            qs[i % 3].dma_start(out=dst, in_=src)
            if i == 4:
                # bv lands before the first v bias-add; bq/bk much later
                nc.scalar.dma_start(
                    out=bv_sb,
                    in_=bass.AP(tensor=bv.tensor, offset=bv.offset,
                                ap=[[0, 128], [1, HDC]]),
                )
            if i == 10:
                nc.scalar.dma_start(
                    out=bq_sb, in_=bq.rearrange("(h p) -> p h", p=128))
                nc.scalar.dma_start(
                    out=bk_sb, in_=bk.rearrange("(h p) -> p h", p=128))
        ones = singles.tile([128, 128], BF16)
        nc.vector.memset(ones, 1.0)
        neg_shift = singles.tile([128, 1], F32)
        nc.vector.memset(neg_shift, -SHIFT)
        woT_sb = singles.tile([128, HPC, H], BF16)
        nc.gpsimd.dma_start(out=woT_sb, in_=woT)

        # persistent activations
        qt_sb = [[persist.tile([128, S], BF16, tag=f"qt{b}{h}", name=f"qt{b}{h}")
                  for h in range(HPC)] for b in range(B)]
        kt_sb = [[persist.tile([128, S], BF16, tag=f"kt{b}{h}", name=f"kt{b}{h}")
                  for h in range(HPC)] for b in range(B)]
        v_sb = [persist.tile([128, TC, HDC], BF16, tag=f"v{b}", name=f"v{b}")
                for b in range(B)]
        aoT_sb = [[persist.tile([128, S], BF16, tag=f"ao{b}{h}", name=f"ao{b}{h}")
                   for h in range(HPC)] for b in range(B)]

        # ---- generators ----
        def gen_qkv(b, half, prefetch=()):
            ht_q = [ht_tiles.pop((b, half, qx)) for qx in range(2)]

            def v_block(g, sub2):
                # two 128-token sub-tiles -> v_sb[:, tt0:tt0+2, :]
                ps = ps_mix.tile([128, 2, HDC], F32, tag="mix",
                                 name=f"v{b}{half}{g}{sub2}")
                for s2 in range(2):
                    sub = sub2 * 2 + s2
                    for fc in range(FC):
                        nc.tensor.matmul(
                            ps[:, s2, :],
                            ht_q[g][:, fc, sub * 128 : (sub + 1) * 128],
                            w_sb["v"][:, fc, :],
                            start=(fc == 0),
                            stop=(fc == FC - 1),
                        )
                tt0 = half * 8 + g * 4 + sub2 * 2
                nc.vector.tensor_add(v_sb[b][:, tt0 : tt0 + 2, :], ps, bv2)

            def qk_block(h, name, dst, bias):
                ps = ps_mix.tile([128, 1024], F32, tag="mix",
                                 name=f"qk{b}{half}{h}{name}")
                for fc in range(FC):
                    lhsT = w_sb[name][:, fc, h * HD : (h + 1) * HD]
                    for n in range(2):
                        nc.tensor.matmul(
                            ps[:, n * 512 : (n + 1) * 512],
                            lhsT,
                            ht_q[n][:, fc, :],
                            start=(fc == 0),
                            stop=(fc == FC - 1),
                        )
                nc.vector.tensor_scalar_add(
                    dst[:, half * 1024 : (half + 1) * 1024], ps,
                    bias[:, h : h + 1],
                )

            # v blocks first (need only wv + their own quarter), qk after
            # (needs wq/wk which arrive later on the weight queues)
            v_block(0, 0)
            yield
            v_block(0, 1)
            yield
            v_block(1, 0)
            yield
            v_block(1, 1)
            yield
            for h in range(HPC):
                qk_block(h, "q", qt_sb[b][h], bq_sb)
                yield
                qk_block(h, "k", kt_sb[b][h], bk_sb)
                yield
            # prefetch at the end: all readers of this half's quarters are
            # emitted, so the ring-buffer WAR dep is recorded
            for nxt in prefetch:
                prefetch_ht(*nxt)

        def emit_av(av, b, h, tcx, pt, nq):
            for n in range(nq):
                nc.tensor.matmul(
                    av[:, n * 512 : (n + 1) * 512],
                    v_sb[b][:, tcx, h * HD : (h + 1) * HD],
                    pt[:, n * 512 : (n + 1) * 512],
                    start=(tcx == 0),
                    stop=(tcx == TC - 1),
                )

        def att_group(b, h, q0, qlen):
            nq = qlen // 512
            av = ps_av.tile([128, qlen], F32, tag="av", name=f"av{b}{h}{q0}")
            pts = []
            quads = []
            octs = []
            for tcx in range(TC):
                ps = ps_mix.tile([128, qlen], F32, tag="mix",
                                 name=f"sc{b}{h}{q0}{tcx}")
                lhsT = kt_sb[b][h][:, tcx * 128 : (tcx + 1) * 128]
                for n in range(nq):
                    nc.tensor.matmul(
                        ps[:, n * 512 : (n + 1) * 512],
                        lhsT,
                        qt_sb[b][h][:, q0 + n * 512 : q0 + (n + 1) * 512],
                        start=True,
                        stop=True,
                    )
                pt = pt_pool.tile([128, qlen], BF16, tag="pt",
                                  name=f"pt{b}{h}{q0}{tcx}")
                nc.scalar.activation(pt, ps, EXP, bias=neg_shift, scale=scale)
                pts.append(pt)
                # AV lags scores by one tcx so it never waits on the exp
                if tcx > 0:
                    emit_av(av, b, h, tcx - 1, pts[tcx - 1], nq)
                if tcx % 4 == 3:
                    pair0 = pair_pool.tile([128, qlen], BF16, tag="pair",
                                           name=f"p0{b}{h}{q0}{tcx}")
                    nc.vector.tensor_add(pair0, pts[-4], pts[-3])
                    pair1 = pair_pool.tile([128, qlen], BF16, tag="pair",
                                           name=f"p1{b}{h}{q0}{tcx}")
                    nc.vector.tensor_add(pair1, pts[-2], pts[-1])
                    quad = quad_pool.tile([128, qlen], BF16, tag="quad",
                                          name=f"q{b}{h}{q0}{tcx}")
                    nc.vector.tensor_add(quad, pair0, pair1)
                    quads.append(quad)
                    if len(quads) % 2 == 0:
                        oct_ = oct_pool.tile([128, qlen], BF16, tag="oct",
                                             name=f"o{b}{h}{q0}{tcx}")
                        nc.vector.tensor_add(oct_, quads[-2], quads[-1])
                        octs.append(oct_)
                yield
            emit_av(av, b, h, TC - 1, pts[TC - 1], nq)
            # yield here: boundary fillers run (and their casts land on the
            # DVE queue) before den/recip/mul, decoupling filler drains from
            # the normalize chain
            yield
            hex_ = hex_pool.tile([128, qlen], BF16, tag="hex",
                                 name=f"hx{b}{h}{q0}")
            nc.vector.tensor_add(hex_, octs[0], octs[1])
            den = ps_mix.tile([128, qlen], F32, tag="mix",
                              name=f"den{b}{h}{q0}")
            for n in range(nq):
                nc.tensor.matmul(
                    den[:, n * 512 : (n + 1) * 512],
                    ones,
                    hex_[:, n * 512 : (n + 1) * 512],
                    start=True,
                    stop=True,
                )
            recip = den_pool.tile([128, qlen], F32, tag="recip",
                                  name=f"r{b}{h}{q0}")
            nc.vector.reciprocal_approx_fast(recip, den)
            nc.vector.tensor_mul(aoT_sb[b][h][:, q0 : q0 + qlen], av, recip)

        def gen_oproj(b, tts, final=False):
            for tt in tts:
                row0 = b * S + tt * 128
                o_full = None
                if final:
                    # latency-critical tail: one contiguous [128, 2048]
                    # store per tt (4KB rows -> large DMA packets)
                    o_full = of_pool.tile([128, 2048], out.dtype, tag="of",
                                          name=f"of{b}{tt}")
                for half2 in range(2):
                    ps = ps_mix.tile([128, 1024], F32, tag="mix",
                                     name=f"o{b}{tt}{half2}")
                    for h in range(HPC):
                        lhsT = aoT_sb[b][h][:, tt * 128 : (tt + 1) * 128]
                        for n in range(2):
                            o0 = half2 * 1024 + n * 512
                            nc.tensor.matmul(
                                ps[:, n * 512 : (n + 1) * 512],
                                lhsT,
                                woT_sb[:, h, o0 : o0 + 512],
                                start=(h == 0),
                                stop=(h == HPC - 1),
                            )
                    if final:
                        dst = o_full[:, half2 * 1024 : (half2 + 1) * 1024]
                        if half2 == 0:
                            nc.vector.tensor_copy(dst, ps)
                        else:
                            nc.scalar.activation(dst, ps, COPY)
                            dma_eng = (nc.sync, nc.scalar)[tt % 2]
                            dma_eng.dma_start(out=out[row0 : row0 + 128, :],
                                              in_=o_full)
                    else:
                        o_tile = o_sb_pool.tile([128, 1024], out.dtype,
                                                tag="o", name=f"ot{b}{tt}{half2}")
                        # all non-final casts on DVE: ScalarE paces the
                        # attention exp stream and must not be stolen from
                        nc.vector.tensor_copy(o_tile, ps)
                        nc.sync.dma_start(
                            out=out[row0 : row0 + 128,
                                    half2 * 1024 : (half2 + 1) * 1024],
                            in_=o_tile,
                        )
                    yield

        # ---- scheduler ----
        fillers = deque()

        def drain(k):
            n = 0
            while fillers and n < k:
                try:
                    next(fillers[0])
                    n += 1
                except StopIteration:
                    fillers.popleft()

        def run_gen(g):
            for _ in g:
                pass

        # phase A: qkv(0) back-to-back (nothing else is ready yet)
        run_gen(gen_qkv(0, 0, prefetch=[(0, 1, 0), (0, 1, 1)]))
        run_gen(gen_qkv(0, 1, prefetch=[(1, 0, 0), (1, 0, 1)]))

        qkv1 = [gen_qkv(1, 0, prefetch=[(1, 1, 0), (1, 1, 1)]),
                gen_qkv(1, 1)]
        fillers.extend(qkv1)
        qkv1_alive = set(qkv1)

        def run_group(b, h, q0, qlen):
            step = 0
            for _ in att_group(b, h, q0, qlen):
                step += 1
                if step % 4 == 0:
                    drain(1)
                elif step == TC + 1:  # pre-den yield at the group boundary
                    drain(2)
            drain(2)

        # spread the b=1 attention groups among the b=0 qh1 groups so the
        # exp/cast load (and oproj readiness) is even across the timeline
        run_group(0, 0, 0, 1024)
        run_group(0, 1, 0, 1024)
        fillers.append(gen_oproj(0, range(0, 8)))
        run_group(0, 0, 1024, 1024)
        # attention(1) emission needs qkv(1) fully emitted
        while any(g in fillers for g in qkv1_alive):
            drain(1)
        run_group(1, 0, 0, 1024)
        run_group(0, 1, 1024, 1024)
        fillers.append(gen_oproj(0, range(8, 16)))
        run_group(1, 1, 0, 1024)
        fillers.append(gen_oproj(1, range(0, 8)))
        run_group(1, 0, 1024, 1024)
        # split the final group so the last o-proj chunk shrinks
        run_group(1, 1, 1024, 512)
        fillers.append(gen_oproj(1, range(8, 12)))
        run_group(1, 1, 1536, 512)
        fillers.append(gen_oproj(1, range(12, 16), final=True))
        drain(10 ** 9)


def kernel(hidden_state, Wq, bq, Wk, bk, Wv, bv, Wo, bo):
    bf16 = ml_dtypes.bfloat16
    h2 = np.asarray(hidden_state, dtype=np.float32).reshape(T, H)
    hT = np.ascontiguousarray(h2.T).astype(bf16)  # [H, T]
    # pre-chunk into SBUF tile layout: [8 quarters, 128, FC, 512]
    hTq = np.ascontiguousarray(
        hT.reshape(FC, 128, 8, 512).transpose(2, 1, 0, 3))

    def w_pre(W, r0):
        # [H, HDC] feature-major -> [128, FC, HDC]
        wT = np.asarray(W, np.float32)[r0 : r0 + HDC, :].T.astype(bf16)
        return np.ascontiguousarray(wT.reshape(FC, 128, HDC).transpose(1, 0, 2))

    in_maps = []
    for c in range(N_CORES):
        r0 = c * HDC
        woT = np.asarray(Wo, np.float32)[:, r0 : r0 + HDC].T.astype(bf16)
        in_maps.append({
            "hTq": hTq,
            "wqT": w_pre(Wq, r0),
            "wkT": w_pre(Wk, r0),
            "wvT": w_pre(Wv, r0),
            "woT": np.ascontiguousarray(
                woT.reshape(HPC, 128, H).transpose(1, 0, 2)),
            "bq": np.asarray(bq, np.float32)[r0 : r0 + HDC].copy(),
            "bk": np.asarray(bk, np.float32)[r0 : r0 + HDC].copy(),
            "bv": np.asarray(bv, np.float32)[r0 : r0 + HDC].reshape(1, HDC).copy(),
        })

    if "nc" not in _CACHE:
        _CACHE["nc"] = build_program()
    nc = _CACHE["nc"]
    _CACHE["in_maps"] = in_maps

    res = run_bass_kernel_spmd(nc, in_maps, core_ids=list(range(N_CORES)))
    total = np.zeros((T, H), np.float32)
    for r in res.results:
        total += np.asarray(r["out"]).astype(np.float32)
    total += np.asarray(bo, np.float32)[None, :]
    return total.reshape(B, S, H)



# revision 12
# speedup vs baseline: 1.0517x; 1.0517x over previous
"""Multi-head attention (B=2, S=2048, H=2048, NH=16) on 8 TRN2 NeuronCores.

Sharding: tensor-parallel over heads - 2 heads per core. Each core computes
q/k/v projections for its heads, per-head attention, and a partial output
projection (its heads' columns of Wo); the host sums the 8 partials.

v3: dependency-ordered fine-grained schedule.
  - qkv emitted as per-block generators (yield every 4 matmuls) in the
    order attention unblocks: v/k/q for (b0,h0) first, so the first
    attention group starts ~55us in (vs ~125us before).
  - Attention groups drain ~1 filler unit per key-tile; group `need` lists
    force specific blocks to finish before a group's scores are emitted.
  - o-proj generators are a low-priority filler queue: drained only when
    qkv fillers are dry, leaving a dense PE-bound o-proj tail.
  - den via pair/quad/oct/hex DVE tree -> single ones-matmul per group.
  - All non-final o-proj psum->sbuf casts on DVE (ScalarE is the exp pacer).
  - hT quarters through a 4-deep pool; b1 quarters prefetched as fillers.
"""

import sys

sys.path.insert(0, "/opt/trn_rl_repo")

from collections import deque
from contextlib import ExitStack

import ml_dtypes
import numpy as np

import concourse.bass as bass
import concourse.tile as tile
from concourse import bacc, mybir
from concourse.bass_utils import run_bass_kernel_spmd

B, S, H, NH = 2, 2048, 2048, 16
HD = H // NH          # 128
N_CORES = 8
HPC = NH // N_CORES   # heads per core = 2
HDC = HPC * HD        # head-dims per core = 256
T = B * S             # 4096 tokens
FC = H // 128         # 16 feature chunks
TC = S // 128         # 16 token tiles per batch
SHIFT = 4.0           # fixed exp shift (softmax-invariant, overflow guard)

BF16 = mybir.dt.bfloat16
F32 = mybir.dt.float32
EXP = mybir.ActivationFunctionType.Exp
COPY = mybir.ActivationFunctionType.Copy

_CACHE = {}


def build_program(out_dtype=BF16):
    nc = bacc.Bacc(
        "TRN2", target_bir_lowering=False, debug=False, num_devices=N_CORES
    )
    # hTq: hT pre-chunked on the host into SBUF tile layout: quarter q
    # (= (b, half, qx)) holds [128, FC, 512] with 16KB contiguous per
    # partition -> one DMA descriptor per partition row.
    hTq = nc.dram_tensor("hTq", [8, 128, FC, 512], BF16, kind="ExternalInput").ap()
    wqT = nc.dram_tensor("wqT", [128, FC, HDC], BF16, kind="ExternalInput").ap()
    wkT = nc.dram_tensor("wkT", [128, FC, HDC], BF16, kind="ExternalInput").ap()
    wvT = nc.dram_tensor("wvT", [128, FC, HDC], BF16, kind="ExternalInput").ap()
    woT = nc.dram_tensor("woT", [128, HPC, H], BF16, kind="ExternalInput").ap()
    bq = nc.dram_tensor("bq", [HDC], F32, kind="ExternalInput").ap()
    bk = nc.dram_tensor("bk", [HDC], F32, kind="ExternalInput").ap()
    bv = nc.dram_tensor("bv", [1, HDC], F32, kind="ExternalInput").ap()
    out = nc.dram_tensor("out", [T, H], out_dtype, kind="ExternalOutput").ap()

    with tile.TileContext(nc) as tc:
        _kernel(tc, out, hTq, wqT, wkT, wvT, woT, bq, bk, bv)
    nc.compile()
    return nc


def _kernel(tc, out, hTq, wqT, wkT, wvT, woT, bq, bk, bv):
    nc = tc.nc
    scale = 1.0 / float(np.sqrt(HD))
    ctx = ExitStack()
    with ctx:
        singles = ctx.enter_context(tc.tile_pool(name="singles", bufs=1))
        persist = ctx.enter_context(tc.tile_pool(name="persist", bufs=1))
        ps_mix = ctx.enter_context(tc.tile_pool(name="ps_mix", bufs=3, space="PSUM"))
        ps_av = ctx.enter_context(tc.tile_pool(name="ps_av", bufs=1, space="PSUM"))
        ht_pool = ctx.enter_context(tc.tile_pool(name="ht", bufs=4))
        pt_pool = ctx.enter_context(tc.tile_pool(name="pt", bufs=9))
        pair_pool = ctx.enter_context(tc.tile_pool(name="pair", bufs=2))
        quad_pool = ctx.enter_context(tc.tile_pool(name="quad", bufs=2))
        oct_pool = ctx.enter_context(tc.tile_pool(name="oct", bufs=2))
        hex_pool = ctx.enter_context(tc.tile_pool(name="hex", bufs=1))
        den_pool = ctx.enter_context(tc.tile_pool(name="den", bufs=1))
        o_sb_pool = ctx.enter_context(tc.tile_pool(name="o_sb", bufs=3))
        of_pool = ctx.enter_context(tc.tile_pool(name="of", bufs=1))

        # ---- hT quarter tiles + prefetch machinery ----
        ht_tiles = {}

        def new_ht_tile(b, half, qx):
            t = ht_pool.tile([128, FC, 512], BF16, tag="ht",
                             name=f"ht{b}{half}{qx}")
            ht_tiles[(b, half, qx)] = t
            return t

        def ht_src(qidx, f0, f1):
            return bass.AP(
                tensor=hTq.tensor,
                offset=hTq.offset + (qidx * 128 * FC + f0) * 512,
                ap=[[FC * 512, 128], [512, f1 - f0], [1, 512]],
            )

        def prefetch_ht(b, half, qx):
            # 4 pieces on sync/gpsimd so the first fc chunks land early
            # (subtile deps let consumers start before the whole tile)
            qidx = b * 4 + half * 2 + qx
            t = new_ht_tile(b, half, qx)
            engs = (nc.sync, nc.gpsimd)
            for g in range(4):
                engs[g % 2].dma_start(out=t[:, 4 * g : 4 * g + 4, :],
                                      in_=ht_src(qidx, 4 * g, 4 * g + 4))

        bq_sb = singles.tile([128, HPC], F32)
        bk_sb = singles.tile([128, HPC], F32)
        # bv broadcast across partitions once ([128, HDC]); the 2-wide group
        # dim is a free-dim stride-0 view at use time
        bv_sb = singles.tile([128, HDC], F32)
        bv2 = bass.AP(tensor=bv_sb.tensor, offset=bv_sb.offset,
                      ap=[bv_sb.ap[0], [0, 2], [1, HDC]])

        w_sb = {}
        for name in ("v", "q", "k"):
            w_sb[name] = singles.tile([128, FC, HDC], BF16, tag=f"w{name}",
                                      name=f"w{name}")

        def w_unit(name, f0, f1):
            src = {"v": wvT, "q": wqT, "k": wkT}[name]
            return (w_sb[name][:, f0:f1, :], src[:, f0:f1, :])

        # ---- initial DMA choreography ----
        # Consumption order: v(0,0,g0) [ht000+wv], v(0,0,g1) [ht001],
        # k(0,0,h0) [wk], q(0,0,h0) [wq], k(0,1,h0) [ht010+ht011].
        # First ht000/wv pieces are 1-fc sized so the PE starts ~8us in.
        ht000 = new_ht_tile(0, 0, 0)
        ht001 = new_ht_tile(0, 0, 1)
        ht010 = new_ht_tile(0, 1, 0)
        ht011 = new_ht_tile(0, 1, 1)
        units = [
            (ht000[:, 0:1, :], ht_src(0, 0, 1)),
            w_unit("v", 0, 1),
            (ht000[:, 1:4, :], ht_src(0, 1, 4)),
            w_unit("v", 1, 4),
        ]
        for g in range(1, 4):
            units.append((ht000[:, 4 * g : 4 * g + 4, :],
                          ht_src(0, 4 * g, 4 * g + 4)))
            units.append(w_unit("v", 4 * g, 4 * g + 4))
        for g in range(4):
            units.append((ht001[:, 4 * g : 4 * g + 4, :],
                          ht_src(1, 4 * g, 4 * g + 4)))
        for g in range(4):
            units.append(w_unit("k", 4 * g, 4 * g + 4))
        for g in range(4):
            units.append(w_unit("q", 4 * g, 4 * g + 4))
        for g in range(4):
            units.append((ht010[:, 4 * g : 4 * g + 4, :],
                          ht_src(2, 4 * g, 4 * g + 4)))
        for g in range(4):
            units.append((ht011[:, 4 * g : 4 * g + 4, :],
                          ht_src(3, 4 * g, 4 * g + 4)))
        qs = (nc.sync, nc.gpsimd, nc.scalar)
        for i, (dst, src) in enumerate(units):
            qs[i % 3].dma_start(out=dst, in_=src)
            if i == 5:
                # bv lands before the first v bias-add
                nc.scalar.dma_start(
                    out=bv_sb,
                    in_=bass.AP(tensor=bv.tensor, offset=bv.offset,
                                ap=[[0, 128], [1, HDC]]),
                )
            if i == 13:
                nc.scalar.dma_start(
                    out=bq_sb, in_=bq.rearrange("(h p) -> p h", p=128))
                nc.scalar.dma_start(
                    out=bk_sb, in_=bk.rearrange("(h p) -> p h", p=128))
        woT_sb = singles.tile([128, HPC, H], BF16)
        nc.gpsimd.dma_start(out=woT_sb, in_=woT)
        ones = singles.tile([128, 128], BF16)
        nc.vector.memset(ones, 1.0)
        neg_shift = singles.tile([128, 1], F32)
        nc.vector.memset(neg_shift, -SHIFT)

        # persistent activations
        qt_sb = [[persist.tile([128, S], BF16, tag=f"qt{b}{h}", name=f"qt{b}{h}")
                  for h in range(HPC)] for b in range(B)]
        kt_sb = [[persist.tile([128, S], BF16, tag=f"kt{b}{h}", name=f"kt{b}{h}")
                  for h in range(HPC)] for b in range(B)]
        v_sb = [persist.tile([128, TC, HDC], BF16, tag=f"v{b}", name=f"v{b}")
                for b in range(B)]
        aoT_sb = [[persist.tile([128, S], BF16, tag=f"ao{b}{h}", name=f"ao{b}{h}")
                   for h in range(HPC)] for b in range(B)]

        # ---- qkv block generators (yield every 4 matmuls) ----
        def gen_v_block(b, half, g, sub2):
            ht_g = ht_tiles[(b, half, g)]
            ps = ps_mix.tile([128, 2, HDC], F32, tag="mix",
                             name=f"v{b}{half}{g}{sub2}")
            n_mm = 0
            for s2 in range(2):
                sub = sub2 * 2 + s2
                for fc in range(FC):
                    nc.tensor.matmul(
                        ps[:, s2, :],
                        ht_g[:, fc, sub * 128 : (sub + 1) * 128],
                        w_sb["v"][:, fc, :],
                        start=(fc == 0),
                        stop=(fc == FC - 1),
                    )
                    n_mm += 1
                    if n_mm % 4 == 0:
                        yield
            tt0 = half * 8 + g * 4 + sub2 * 2
            nc.vector.tensor_add(v_sb[b][:, tt0 : tt0 + 2, :], ps, bv2)
            yield

        def gen_qk_block(b, half, h, name):
            dst = (qt_sb if name == "q" else kt_sb)[b][h]
            bias = bq_sb if name == "q" else bk_sb
            ps = ps_mix.tile([128, 1024], F32, tag="mix",
                             name=f"qk{b}{half}{h}{name}")
            n_mm = 0
            for fc in range(FC):
                lhsT = w_sb[name][:, fc, h * HD : (h + 1) * HD]
                for n in range(2):
                    nc.tensor.matmul(
                        ps[:, n * 512 : (n + 1) * 512],
                        lhsT,
                        ht_tiles[(b, half, n)][:, fc, :],
                        start=(fc == 0),
                        stop=(fc == FC - 1),
                    )
                    n_mm += 1
                    if n_mm % 4 == 0:
                        yield
            nc.vector.tensor_scalar_add(
                dst[:, half * 1024 : (half + 1) * 1024], ps,
                bias[:, h : h + 1],
            )
            yield

        def gen_prefetch(*quarters):
            for qtr in quarters:
                prefetch_ht(*qtr)
            yield

        # ---- attention ----
        def emit_av(av, b, h, tcx, pt, nq):
            for n in range(nq):
                nc.tensor.matmul(
                    av[:, n * 512 : (n + 1) * 512],
                    v_sb[b][:, tcx, h * HD : (h + 1) * HD],
                    pt[:, n * 512 : (n + 1) * 512],
                    start=(tcx == 0),
                    stop=(tcx == TC - 1),
                )

        def att_group(b, h, q0, qlen, ensure_v):
            nq = qlen // 512
            av = ps_av.tile([128, qlen], F32, tag="av", name=f"av{b}{h}{q0}")
            pts = []
            quads = []
            octs = []
            for tcx in range(TC):
                ps = ps_mix.tile([128, qlen], F32, tag="mix",
                                 name=f"sc{b}{h}{q0}{tcx}")
                lhsT = kt_sb[b][h][:, tcx * 128 : (tcx + 1) * 128]
                for n in range(nq):
                    nc.tensor.matmul(
                        ps[:, n * 512 : (n + 1) * 512],
                        lhsT,
                        qt_sb[b][h][:, q0 + n * 512 : q0 + (n + 1) * 512],
                        start=True,
                        stop=True,
                    )
                pt = pt_pool.tile([128, qlen], BF16, tag="pt",
                                  name=f"pt{b}{h}{q0}{tcx}")
                nc.scalar.activation(pt, ps, EXP, bias=neg_shift, scale=scale)
                pts.append(pt)
                # AV lags scores by one tcx so it never waits on the exp.
                # ensure_v force-drains the filler block producing v tile
                # tcx-1 before the AV matmul is emitted: the PE runs its
                # queue in order, so emitting AV ahead of its v-producing
                # matmuls would deadlock.
                if tcx > 0:
                    ensure_v(b, tcx - 1)
                    emit_av(av, b, h, tcx - 1, pts[tcx - 1], nq)
                if tcx % 4 == 3:
                    pair0 = pair_pool.tile([128, qlen], BF16, tag="pair",
                                           name=f"p0{b}{h}{q0}{tcx}")
                    nc.vector.tensor_add(pair0, pts[-4], pts[-3])
                    pair1 = pair_pool.tile([128, qlen], BF16, tag="pair",
                                           name=f"p1{b}{h}{q0}{tcx}")
                    nc.vector.tensor_add(pair1, pts[-2], pts[-1])
                    quad = quad_pool.tile([128, qlen], BF16, tag="quad",
                                          name=f"q{b}{h}{q0}{tcx}")
                    nc.vector.tensor_add(quad, pair0, pair1)
                    quads.append(quad)
                    if len(quads) % 2 == 0:
                        oct_ = oct_pool.tile([128, qlen], BF16, tag="oct",
                                             name=f"o{b}{h}{q0}{tcx}")
                        nc.vector.tensor_add(oct_, quads[-2], quads[-1])
                        octs.append(oct_)
                yield
            ensure_v(b, TC - 1)
            emit_av(av, b, h, TC - 1, pts[TC - 1], nq)
            # boundary yield: fillers run before the normalize chain
            yield
            hex_ = hex_pool.tile([128, qlen], BF16, tag="hex",
                                 name=f"hx{b}{h}{q0}")
            nc.vector.tensor_add(hex_, octs[0], octs[1])
            den = ps_mix.tile([128, qlen], F32, tag="mix",
                              name=f"den{b}{h}{q0}")
            for n in range(nq):
                nc.tensor.matmul(
                    den[:, n * 512 : (n + 1) * 512],
                    ones,
                    hex_[:, n * 512 : (n + 1) * 512],
                    start=True,
                    stop=True,
                )
            recip = den_pool.tile([128, qlen], F32, tag="recip",
                                  name=f"r{b}{h}{q0}")
            nc.vector.reciprocal_approx_fast(recip, den)
            nc.vector.tensor_mul(aoT_sb[b][h][:, q0 : q0 + qlen], av, recip)

        # ---- output projection ----
        def gen_oproj(b, tts, final=False):
            for tt in tts:
                row0 = b * S + tt * 128
                o_full = None
                if final:
                    # latency-critical tail: one contiguous [128, 2048]
                    # store per tt (4KB rows -> large DMA packets)
                    o_full = of_pool.tile([128, 2048], out.dtype, tag="of",
                                          name=f"of{b}{tt}")
                for half2 in range(2):
                    ps = ps_mix.tile([128, 1024], F32, tag="mix",
                                     name=f"o{b}{tt}{half2}")
                    for h in range(HPC):
                        lhsT = aoT_sb[b][h][:, tt * 128 : (tt + 1) * 128]
                        for n in range(2):
                            o0 = half2 * 1024 + n * 512
                            nc.tensor.matmul(
                                ps[:, n * 512 : (n + 1) * 512],
                                lhsT,
                                woT_sb[:, h, o0 : o0 + 512],
                                start=(h == 0),
                                stop=(h == HPC - 1),
                            )
                    if final:
                        dst = o_full[:, half2 * 1024 : (half2 + 1) * 1024]
                        if half2 == 0:
                            nc.vector.tensor_copy(dst, ps)
                        else:
                            nc.scalar.activation(dst, ps, COPY)
                            dma_eng = (nc.sync, nc.scalar)[tt % 2]
                            dma_eng.dma_start(out=out[row0 : row0 + 128, :],
                                              in_=o_full)
                    else:
                        o_tile = o_sb_pool.tile([128, 1024], out.dtype,
                                                tag="o", name=f"ot{b}{tt}{half2}")
                        # all non-final casts on DVE: ScalarE paces the exp
                        # stream and must not be stolen from
                        nc.vector.tensor_copy(o_tile, ps)
                        nc.sync.dma_start(
                            out=out[row0 : row0 + 128,
                                    half2 * 1024 : (half2 + 1) * 1024],
                            in_=o_tile,
                        )
                    yield

        # ---- scheduler ----
        fillers = deque()
        lowprio = deque()
        live = set()
        vmap = {}  # (b, token_tile) -> filler gen producing that v tile

        def enq(gen):
            fillers.append(gen)
            live.add(gen)
            return gen

        def enq_v(b, half, g, s2):
            gen = enq(gen_v_block(b, half, g, s2))
            tt0 = half * 8 + g * 4 + s2 * 2
            vmap[(b, tt0)] = vmap[(b, tt0 + 1)] = gen
            return gen

        def drain(k):
            n = 0
            while n < k:
                if fillers:
                    q = fillers
                elif lowprio:
                    q = lowprio
                else:
                    return
                try:
                    next(q[0])
                    n += 1
                except StopIteration:
                    live.discard(q[0])
                    q.popleft()

        def ensure_v(b, tcx):
            g = vmap.get((b, tcx))
            while g is not None and g in live:
                drain(1)

        def run_gen(g):
            for _ in g:
                pass

        def run_group(b, h, q0, qlen, need=()):
            for g in need:
                while g in live:
                    drain(1)
            step = 0
            for _ in att_group(b, h, q0, qlen, ensure_v):
                step += 1
                drain(1)
                if step == TC + 1:
                    drain(2)
            drain(2)

        # phase A: minimal serial prefix for the first attention group
        run_gen(gen_v_block(0, 0, 0, 0))
        run_gen(gen_v_block(0, 0, 0, 1))
        run_gen(gen_v_block(0, 0, 1, 0))
        run_gen(gen_v_block(0, 0, 1, 1))
        run_gen(gen_qk_block(0, 0, 0, "k"))
        run_gen(gen_qk_block(0, 0, 0, "q"))
        run_gen(gen_qk_block(0, 1, 0, "k"))

        # fillers in dependency order
        for g in range(2):
            for s2 in range(2):
                enq_v(0, 1, g, s2)
        k001 = enq(gen_qk_block(0, 0, 1, "k"))
        k011 = enq(gen_qk_block(0, 1, 1, "k"))
        q001 = enq(gen_qk_block(0, 0, 1, "q"))
        enq(gen_prefetch((1, 0, 0), (1, 0, 1)))
        q010 = enq(gen_qk_block(0, 1, 0, "q"))
        q011 = enq(gen_qk_block(0, 1, 1, "q"))
        enq(gen_prefetch((1, 1, 0), (1, 1, 1)))
        for g in range(2):
            for s2 in range(2):
                enq_v(1, 0, g, s2)
        k100 = enq(gen_qk_block(1, 0, 0, "k"))
        q100 = enq(gen_qk_block(1, 0, 0, "q"))
        k110 = enq(gen_qk_block(1, 1, 0, "k"))
        for g in range(2):
            for s2 in range(2):
                enq_v(1, 1, g, s2)
        k101 = enq(gen_qk_block(1, 0, 1, "k"))
        k111 = enq(gen_qk_block(1, 1, 1, "k"))
        q101 = enq(gen_qk_block(1, 0, 1, "q"))
        q110 = enq(gen_qk_block(1, 1, 0, "q"))
        q111 = enq(gen_qk_block(1, 1, 1, "q"))

        run_group(0, 0, 0, 1024)
        run_group(0, 1, 0, 1024, need=[k001, k011, q001])
        lowprio.append(gen_oproj(0, range(0, 8)))
        run_group(0, 0, 1024, 1024, need=[q010])
        run_group(0, 1, 1024, 1024, need=[q011])
        lowprio.append(gen_oproj(0, range(8, 16)))
        run_group(1, 0, 0, 1024, need=[k100, q100, k110])
        run_group(1, 1, 0, 1024, need=[k101, k111, q101])
        lowprio.append(gen_oproj(1, range(0, 8)))
        run_group(1, 0, 1024, 1024, need=[q110])
        run_group(1, 1, 1024, 512, need=[q111])
        lowprio.append(gen_oproj(1, range(8, 12)))
        run_group(1, 1, 1536, 512)
        lowprio.append(gen_oproj(1, range(12, 16), final=True))
        drain(10 ** 9)


def kernel(hidden_state, Wq, bq, Wk, bk, Wv, bv, Wo, bo):
    bf16 = ml_dtypes.bfloat16
    h2 = np.asarray(hidden_state, dtype=np.float32).reshape(T, H)
    hT = np.ascontiguousarray(h2.T).astype(bf16)  # [H, T]
    # pre-chunk into SBUF tile layout: [8 quarters, 128, FC, 512]
    hTq = np.ascontiguousarray(
        hT.reshape(FC, 128, 8, 512).transpose(2, 1, 0, 3))

    def w_pre(W, r0):
        # [H, HDC] feature-major -> [128, FC, HDC]
        wT = np.asarray(W, np.float32)[r0 : r0 + HDC, :].T.astype(bf16)
        return np.ascontiguousarray(wT.reshape(FC, 128, HDC).transpose(1, 0, 2))

    in_maps = []
    for c in range(N_CORES):
        r0 = c * HDC
        woT = np.asarray(Wo, np.float32)[:, r0 : r0 + HDC].T.astype(bf16)
        in_maps.append({
            "hTq": hTq,
            "wqT": w_pre(Wq, r0),
            "wkT": w_pre(Wk, r0),
            "wvT": w_pre(Wv, r0),
            "woT": np.ascontiguousarray(
                woT.reshape(HPC, 128, H).transpose(1, 0, 2)),
            "bq": np.asarray(bq, np.float32)[r0 : r0 + HDC].copy(),
            "bk": np.asarray(bk, np.float32)[r0 : r0 + HDC].copy(),
            "bv": np.asarray(bv, np.float32)[r0 : r0 + HDC].reshape(1, HDC).copy(),
        })

    if "nc" not in _CACHE:
        _CACHE["nc"] = build_program()
    nc = _CACHE["nc"]
    _CACHE["in_maps"] = in_maps

    res = run_bass_kernel_spmd(nc, in_maps, core_ids=list(range(N_CORES)))
    total = np.zeros((T, H), np.float32)
    for r in res.results:
        total += np.asarray(r["out"]).astype(np.float32)
    total += np.asarray(bo, np.float32)[None, :]
    return total.reshape(B, S, H)


# revision 16
# speedup vs baseline: 1.1138x; 1.0590x over previous
"""Multi-head attention (B=2, S=2048, H=2048, NH=16) on 8 TRN2 NeuronCores.

Sharding: tensor-parallel over heads - 2 heads per core. Each core computes
q/k/v projections for its heads, per-head attention, and a partial output
projection (its heads' columns of Wo); the host sums the 8 partials.

v3: dependency-ordered fine-grained schedule.
  - qkv emitted as per-block generators (yield every 4 matmuls) in the
    order attention unblocks: v/k/q for (b0,h0) first, so the first
    attention group starts ~55us in (vs ~125us before).
  - Attention groups drain ~1 filler unit per key-tile; group `need` lists
    force specific blocks to finish before a group's scores are emitted.
  - o-proj generators are a low-priority filler queue: drained only when
    qkv fillers are dry, leaving a dense PE-bound o-proj tail.
  - den via pair/quad/oct/hex DVE tree -> single ones-matmul per group.
  - All non-final o-proj psum->sbuf casts on DVE (ScalarE is the exp pacer).
  - hT quarters through a 4-deep pool; b1 quarters prefetched as fillers.
"""

import sys

sys.path.insert(0, "/opt/trn_rl_repo")

from collections import deque
from contextlib import ExitStack

import ml_dtypes
import numpy as np

import concourse.bass as bass
import concourse.tile as tile
from concourse import bacc, mybir
from concourse.bass_utils import run_bass_kernel_spmd

B, S, H, NH = 2, 2048, 2048, 16
HD = H // NH          # 128
N_CORES = 8
HPC = NH // N_CORES   # heads per core = 2
HDC = HPC * HD        # head-dims per core = 256
T = B * S             # 4096 tokens
FC = H // 128         # 16 feature chunks
TC = S // 128         # 16 token tiles per batch
SHIFT = 4.0           # fixed exp shift (softmax-invariant, overflow guard)

BF16 = mybir.dt.bfloat16
F32 = mybir.dt.float32
EXP = mybir.ActivationFunctionType.Exp
COPY = mybir.ActivationFunctionType.Copy

_CACHE = {}


def build_program(out_dtype=BF16):
    nc = bacc.Bacc(
        "TRN2", target_bir_lowering=False, debug=False, num_devices=N_CORES
    )
    # hTq: hT pre-chunked on the host into SBUF tile layout: quarter q
    # (= (b, half, qx)) holds [128, FC, 512] with 16KB contiguous per
    # partition -> one DMA descriptor per partition row.
    hTq = nc.dram_tensor("hTq", [8, 128, FC, 512], BF16, kind="ExternalInput").ap()
    wqT = nc.dram_tensor("wqT", [128, FC, HDC], BF16, kind="ExternalInput").ap()
    wkT = nc.dram_tensor("wkT", [128, FC, HDC], BF16, kind="ExternalInput").ap()
    wvT = nc.dram_tensor("wvT", [128, FC, HDC], BF16, kind="ExternalInput").ap()
    woT = nc.dram_tensor("woT", [128, HPC, H], BF16, kind="ExternalInput").ap()
    bq = nc.dram_tensor("bq", [HDC], F32, kind="ExternalInput").ap()
    bk = nc.dram_tensor("bk", [HDC], F32, kind="ExternalInput").ap()
    bv = nc.dram_tensor("bv", [1, HDC], F32, kind="ExternalInput").ap()
    out = nc.dram_tensor("out", [T, H], out_dtype, kind="ExternalOutput").ap()

    with tile.TileContext(nc) as tc:
        _kernel(tc, out, hTq, wqT, wkT, wvT, woT, bq, bk, bv)
    nc.compile()
    return nc


def _kernel(tc, out, hTq, wqT, wkT, wvT, woT, bq, bk, bv):
    nc = tc.nc
    scale = 1.0 / float(np.sqrt(HD))
    ctx = ExitStack()
    with ctx:
        singles = ctx.enter_context(tc.tile_pool(name="singles", bufs=1))
        persist = ctx.enter_context(tc.tile_pool(name="persist", bufs=1))
        ps_mix = ctx.enter_context(tc.tile_pool(name="ps_mix", bufs=3, space="PSUM"))
        ps_av = ctx.enter_context(tc.tile_pool(name="ps_av", bufs=1, space="PSUM"))
        ht_pool = ctx.enter_context(tc.tile_pool(name="ht", bufs=4))
        pt_pool = ctx.enter_context(tc.tile_pool(name="pt", bufs=9))
        quad_pool = ctx.enter_context(tc.tile_pool(name="quad", bufs=2))
        oct_pool = ctx.enter_context(tc.tile_pool(name="oct", bufs=2))
        hex_pool = ctx.enter_context(tc.tile_pool(name="hex", bufs=1))
        den_pool = ctx.enter_context(tc.tile_pool(name="den", bufs=1))
        o_sb_pool = ctx.enter_context(tc.tile_pool(name="o_sb", bufs=3))
        of_pool = ctx.enter_context(tc.tile_pool(name="of", bufs=2))

        # ---- hT quarter tiles + prefetch machinery ----
        ht_tiles = {}

        def new_ht_tile(b, half, qx):
            t = ht_pool.tile([128, FC, 512], BF16, tag="ht",
                             name=f"ht{b}{half}{qx}")
            ht_tiles[(b, half, qx)] = t
            return t

        def ht_src(qidx, f0, f1):
            return bass.AP(
                tensor=hTq.tensor,
                offset=hTq.offset + (qidx * 128 * FC + f0) * 512,
                ap=[[FC * 512, 128], [512, f1 - f0], [1, 512]],
            )

        def prefetch_ht(b, half, qx):
            # 4 pieces on sync/gpsimd so the first fc chunks land early
            # (subtile deps let consumers start before the whole tile)
            qidx = b * 4 + half * 2 + qx
            t = new_ht_tile(b, half, qx)
            engs = (nc.sync, nc.gpsimd)
            for g in range(4):
                engs[g % 2].dma_start(out=t[:, 4 * g : 4 * g + 4, :],
                                      in_=ht_src(qidx, 4 * g, 4 * g + 4))

        bq_sb = singles.tile([128, HPC], F32)
        bk_sb = singles.tile([128, HPC], F32)
        # bv broadcast across partitions once ([128, HDC]); the 2-wide group
        # dim is a free-dim stride-0 view at use time
        bv_sb = singles.tile([128, HDC], F32)
        bv2 = bass.AP(tensor=bv_sb.tensor, offset=bv_sb.offset,
                      ap=[bv_sb.ap[0], [0, 2], [1, HDC]])

        w_sb = {}
        for name in ("v", "q", "k"):
            w_sb[name] = singles.tile([128, FC, HDC], BF16, tag=f"w{name}",
                                      name=f"w{name}")

        def w_unit(name, f0, f1):
            src = {"v": wvT, "q": wqT, "k": wkT}[name]
            return (w_sb[name][:, f0:f1, :], src[:, f0:f1, :])

        # ---- initial DMA choreography ----
        # Consumption order: v(0,0,g0) [ht000+wv], v(0,0,g1) [ht001],
        # k(0,0,h0) [wk], q(0,0,h0) [wq], k(0,1,h0) [ht010+ht011].
        # First ht000/wv pieces are 1-fc sized so the PE starts ~8us in.
        ht000 = new_ht_tile(0, 0, 0)
        ht001 = new_ht_tile(0, 0, 1)
        ht010 = new_ht_tile(0, 1, 0)
        ht011 = new_ht_tile(0, 1, 1)
        units = [
            (ht000[:, 0:1, :], ht_src(0, 0, 1)),
            w_unit("v", 0, 1),
            (ht000[:, 1:4, :], ht_src(0, 1, 4)),
            w_unit("v", 1, 4),
        ]
        for g in range(1, 4):
            units.append((ht000[:, 4 * g : 4 * g + 4, :],
                          ht_src(0, 4 * g, 4 * g + 4)))
            units.append(w_unit("v", 4 * g, 4 * g + 4))
        for g in range(4):
            units.append((ht001[:, 4 * g : 4 * g + 4, :],
                          ht_src(1, 4 * g, 4 * g + 4)))
        for g in range(4):
            units.append(w_unit("k", 4 * g, 4 * g + 4))
        for g in range(4):
            units.append(w_unit("q", 4 * g, 4 * g + 4))
        for g in range(4):
            units.append((ht010[:, 4 * g : 4 * g + 4, :],
                          ht_src(2, 4 * g, 4 * g + 4)))
        for g in range(4):
            units.append((ht011[:, 4 * g : 4 * g + 4, :],
                          ht_src(3, 4 * g, 4 * g + 4)))
        # greedy by-bytes queue assignment: units are listed in consumption
        # order; keeping per-queue byte loads balanced makes arrival order
        # track consumption order (round-robin by index does not, since
        # unit sizes differ)
        qs = (nc.sync, nc.gpsimd, nc.scalar)
        qbytes = [0, 0, 0]
        for i, (dst, src) in enumerate(units):
            qi = qbytes.index(min(qbytes))
            qs[qi].dma_start(out=dst, in_=src)
            qbytes[qi] += dst.size() * 2
            if i == 5:
                # bv lands before the first v bias-add
                nc.scalar.dma_start(
                    out=bv_sb,
                    in_=bass.AP(tensor=bv.tensor, offset=bv.offset,
                                ap=[[0, 128], [1, HDC]]),
                )
            if i == 13:
                nc.scalar.dma_start(
                    out=bq_sb, in_=bq.rearrange("(h p) -> p h", p=128))
                nc.scalar.dma_start(
                    out=bk_sb, in_=bk.rearrange("(h p) -> p h", p=128))
        woT_sb = singles.tile([128, HPC, H], BF16)
        nc.sync.dma_start(out=woT_sb[:, 0, :], in_=woT[:, 0, :])
        nc.gpsimd.dma_start(out=woT_sb[:, 1, :], in_=woT[:, 1, :])
        ones = singles.tile([128, 128], BF16)
        nc.vector.memset(ones, 1.0)
        neg_shift = singles.tile([128, 1], F32)
        nc.vector.memset(neg_shift, -SHIFT)

        # persistent activations
        qt_sb = [[persist.tile([128, S], BF16, tag=f"qt{b}{h}", name=f"qt{b}{h}")
                  for h in range(HPC)] for b in range(B)]
        kt_sb = [[persist.tile([128, S], BF16, tag=f"kt{b}{h}", name=f"kt{b}{h}")
                  for h in range(HPC)] for b in range(B)]
        v_sb = [persist.tile([128, TC, HDC], BF16, tag=f"v{b}", name=f"v{b}")
                for b in range(B)]
        aoT_sb = [[persist.tile([128, S], BF16, tag=f"ao{b}{h}", name=f"ao{b}{h}")
                   for h in range(HPC)] for b in range(B)]

        # ---- qkv block generators (yield every 4 matmuls) ----
        def gen_v_block(b, half, g, sub2):
            ht_g = ht_tiles[(b, half, g)]
            ps = ps_mix.tile([128, 2, HDC], F32, tag="mix",
                             name=f"v{b}{half}{g}{sub2}")
            n_mm = 0
            for s2 in range(2):
                sub = sub2 * 2 + s2
                for fc in range(FC):
                    nc.tensor.matmul(
                        ps[:, s2, :],
                        ht_g[:, fc, sub * 128 : (sub + 1) * 128],
                        w_sb["v"][:, fc, :],
                        start=(fc == 0),
                        stop=(fc == FC - 1),
                    )
                    n_mm += 1
                    if n_mm % 4 == 0:
                        yield
            tt0 = half * 8 + g * 4 + sub2 * 2
            nc.vector.tensor_add(v_sb[b][:, tt0 : tt0 + 2, :], ps, bv2)
            yield

        def gen_qk_block(b, half, h, name):
            dst = (qt_sb if name == "q" else kt_sb)[b][h]
            bias = bq_sb if name == "q" else bk_sb
            ps = ps_mix.tile([128, 1024], F32, tag="mix",
                             name=f"qk{b}{half}{h}{name}")
            n_mm = 0
            for fc in range(FC):
                lhsT = w_sb[name][:, fc, h * HD : (h + 1) * HD]
                for n in range(2):
                    nc.tensor.matmul(
                        ps[:, n * 512 : (n + 1) * 512],
                        lhsT,
                        ht_tiles[(b, half, n)][:, fc, :],
                        start=(fc == 0),
                        stop=(fc == FC - 1),
                    )
                    n_mm += 1
                    if n_mm % 4 == 0:
                        yield
            nc.vector.tensor_scalar_add(
                dst[:, half * 1024 : (half + 1) * 1024], ps,
                bias[:, h : h + 1],
            )
            yield

        def gen_prefetch(*quarters):
            for qtr in quarters:
                prefetch_ht(*qtr)
            yield

        # ---- attention ----
        def emit_av(av, b, h, tcx, pt, nq):
            for n in range(nq):
                nc.tensor.matmul(
                    av[:, n * 512 : (n + 1) * 512],
                    v_sb[b][:, tcx, h * HD : (h + 1) * HD],
                    pt[:, n * 512 : (n + 1) * 512],
                    start=(tcx == 0),
                    stop=(tcx == TC - 1),
                )

        def att_group(b, h, q0, qlen, ensure_v):
            nq = qlen // 512
            av = ps_av.tile([128, qlen], F32, tag="av", name=f"av{b}{h}{q0}")
            pts = []
            quads = []
            octs = []
            for tcx in range(TC):
                ps = ps_mix.tile([128, qlen], F32, tag="mix",
                                 name=f"sc{b}{h}{q0}{tcx}")
                lhsT = kt_sb[b][h][:, tcx * 128 : (tcx + 1) * 128]
                for n in range(nq):
                    nc.tensor.matmul(
                        ps[:, n * 512 : (n + 1) * 512],
                        lhsT,
                        qt_sb[b][h][:, q0 + n * 512 : q0 + (n + 1) * 512],
                        start=True,
                        stop=True,
                    )
                pt = pt_pool.tile([128, qlen], BF16, tag="pt",
                                  name=f"pt{b}{h}{q0}{tcx}")
                nc.scalar.activation(pt, ps, EXP, bias=neg_shift, scale=scale)
                pts.append(pt)
                # AV lags scores by one tcx so it never waits on the exp.
                # ensure_v force-drains the filler block producing v tile
                # tcx-1 before the AV matmul is emitted: the PE runs its
                # queue in order, so emitting AV ahead of its v-producing
                # matmuls would deadlock.
                if tcx > 0:
                    ensure_v(b, tcx - 1)
                    emit_av(av, b, h, tcx - 1, pts[tcx - 1], nq)
                # incremental quad chain: one DVE add per tcx (tcx%4!=0),
                # so after the last exp only quad3+oct1+hex remain -> the
                # den matmul's wait on the DVE chain shrinks to ~3us
                if tcx % 4 == 1:
                    quad = quad_pool.tile([128, qlen], BF16, tag="quad",
                                          name=f"q{b}{h}{q0}{tcx}")
                    nc.vector.tensor_add(quad, pts[-2], pts[-1])
                    quads.append(quad)
                elif tcx % 4 in (2, 3):
                    nc.vector.tensor_add(quads[-1], quads[-1], pts[-1])
                    if tcx % 4 == 3 and len(quads) % 2 == 0:
                        oct_ = oct_pool.tile([128, qlen], BF16, tag="oct",
                                             name=f"o{b}{h}{q0}{tcx}")
                        nc.vector.tensor_add(oct_, quads[-2], quads[-1])
                        octs.append(oct_)
                yield
            ensure_v(b, TC - 1)
            emit_av(av, b, h, TC - 1, pts[TC - 1], nq)
            # boundary yield: fillers run before the normalize chain
            yield
            hex_ = hex_pool.tile([128, qlen], BF16, tag="hex",
                                 name=f"hx{b}{h}{q0}")
            nc.vector.tensor_add(hex_, octs[0], octs[1])
            den = ps_mix.tile([128, qlen], F32, tag="mix",
                              name=f"den{b}{h}{q0}")
            for n in range(nq):
                nc.tensor.matmul(
                    den[:, n * 512 : (n + 1) * 512],
                    ones,
                    hex_[:, n * 512 : (n + 1) * 512],
                    start=True,
                    stop=True,
                )
            recip = den_pool.tile([128, qlen], F32, tag="recip",
                                  name=f"r{b}{h}{q0}")
            nc.vector.reciprocal_approx_fast(recip, den)
            nc.vector.tensor_mul(aoT_sb[b][h][:, q0 : q0 + qlen], av, recip)

        # ---- output projection ----
        def gen_oproj(b, tts, final=False):
            for tt in tts:
                row0 = b * S + tt * 128
                o_full = None
                if final:
                    # latency-critical tail: one contiguous [128, 2048]
                    # store per tt (4KB rows -> large DMA packets)
                    o_full = of_pool.tile([128, 2048], out.dtype, tag="of",
                                          name=f"of{b}{tt}")
                for half2 in range(2):
                    ps = ps_mix.tile([128, 1024], F32, tag="mix",
                                     name=f"o{b}{tt}{half2}")
                    for h in range(HPC):
                        lhsT = aoT_sb[b][h][:, tt * 128 : (tt + 1) * 128]
                        for n in range(2):
                            o0 = half2 * 1024 + n * 512
                            nc.tensor.matmul(
                                ps[:, n * 512 : (n + 1) * 512],
                                lhsT,
                                woT_sb[:, h, o0 : o0 + 512],
                                start=(h == 0),
                                stop=(h == HPC - 1),
                            )
                    if final:
                        dst = o_full[:, half2 * 1024 : (half2 + 1) * 1024]
                        if half2 == 0:
                            nc.vector.tensor_copy(dst, ps)
                        else:
                            nc.scalar.activation(dst, ps, COPY)
                            dma_eng = (nc.sync, nc.scalar)[tt % 2]
                            dma_eng.dma_start(out=out[row0 : row0 + 128, :],
                                              in_=o_full)
                    else:
                        o_tile = o_sb_pool.tile([128, 1024], out.dtype,
                                                tag="o", name=f"ot{b}{tt}{half2}")
                        # all non-final casts on DVE: ScalarE paces the exp
                        # stream and must not be stolen from
                        nc.vector.tensor_copy(o_tile, ps)
                        nc.sync.dma_start(
                            out=out[row0 : row0 + 128,
                                    half2 * 1024 : (half2 + 1) * 1024],
                            in_=o_tile,
                        )
                    yield

        # ---- scheduler ----
        fillers = deque()
        lowprio = deque()
        live = set()
        vmap = {}  # (b, token_tile) -> filler gen producing that v tile

        def enq(gen):
            fillers.append(gen)
            live.add(gen)
            return gen

        def enq_v(b, half, g, s2):
            gen = enq(gen_v_block(b, half, g, s2))
            tt0 = half * 8 + g * 4 + s2 * 2
            vmap[(b, tt0)] = vmap[(b, tt0 + 1)] = gen
            return gen

        def drain(k):
            n = 0
            while n < k:
                if fillers:
                    q = fillers
                elif lowprio:
                    q = lowprio
                else:
                    return
                try:
                    next(q[0])
                    n += 1
                except StopIteration:
                    live.discard(q[0])
                    q.popleft()

        def ensure_v(b, tcx):
            g = vmap.get((b, tcx))
            while g is not None and g in live:
                drain(1)

        def run_gen(g):
            for _ in g:
                pass

        def run_group(b, h, q0, qlen, need=()):
            for g in need:
                while g in live:
                    drain(1)
            step = 0
            for _ in att_group(b, h, q0, qlen, ensure_v):
                step += 1
                drain(1)
                if step == TC + 1:
                    # boundary: fill the den-matmul's wait on the DVE
                    # quad/oct/hex chain with ~5us of filler work
                    drain(5)
            drain(2)

        # phase A: minimal serial prefix for the first attention group
        run_gen(gen_v_block(0, 0, 0, 0))
        run_gen(gen_v_block(0, 0, 0, 1))
        run_gen(gen_v_block(0, 0, 1, 0))
        run_gen(gen_v_block(0, 0, 1, 1))
        run_gen(gen_qk_block(0, 0, 0, "k"))
        run_gen(gen_qk_block(0, 0, 0, "q"))
        run_gen(gen_qk_block(0, 1, 0, "k"))

        # fillers in dependency order
        for g in range(2):
            for s2 in range(2):
                enq_v(0, 1, g, s2)
        k001 = enq(gen_qk_block(0, 0, 1, "k"))
        k011 = enq(gen_qk_block(0, 1, 1, "k"))
        q001 = enq(gen_qk_block(0, 0, 1, "q"))
        enq(gen_prefetch((1, 0, 0), (1, 0, 1)))
        q010 = enq(gen_qk_block(0, 1, 0, "q"))
        q011 = enq(gen_qk_block(0, 1, 1, "q"))
        enq(gen_prefetch((1, 1, 0), (1, 1, 1)))
        for g in range(2):
            for s2 in range(2):
                enq_v(1, 0, g, s2)
        k100 = enq(gen_qk_block(1, 0, 0, "k"))
        q100 = enq(gen_qk_block(1, 0, 0, "q"))
        k110 = enq(gen_qk_block(1, 1, 0, "k"))
        for g in range(2):
            for s2 in range(2):
                enq_v(1, 1, g, s2)
        k101 = enq(gen_qk_block(1, 0, 1, "k"))
        k111 = enq(gen_qk_block(1, 1, 1, "k"))
        q101 = enq(gen_qk_block(1, 0, 1, "q"))
        q110 = enq(gen_qk_block(1, 1, 0, "q"))
        q111 = enq(gen_qk_block(1, 1, 1, "q"))

        run_group(0, 0, 0, 1024)
        run_group(0, 1, 0, 1024, need=[k001, k011, q001])
        lowprio.append(gen_oproj(0, range(0, 8)))
        run_group(0, 0, 1024, 1024, need=[q010])
        run_group(0, 1, 1024, 1024, need=[q011])
        lowprio.append(gen_oproj(0, range(8, 16)))
        run_group(1, 0, 0, 1024, need=[k100, q100, k110])
        run_group(1, 1, 0, 1024, need=[k101, k111, q101])
        lowprio.append(gen_oproj(1, range(0, 8)))
        run_group(1, 0, 1024, 1024, need=[q110])
        run_group(1, 1, 1024, 512, need=[q111])
        lowprio.append(gen_oproj(1, range(8, 12)))
        run_group(1, 1, 1536, 512)
        lowprio.append(gen_oproj(1, range(12, 16), final=True))
        drain(10 ** 9)


def kernel(hidden_state, Wq, bq, Wk, bk, Wv, bv, Wo, bo):
    bf16 = ml_dtypes.bfloat16
    h2 = np.asarray(hidden_state, dtype=np.float32).reshape(T, H)
    hT = np.ascontiguousarray(h2.T).astype(bf16)  # [H, T]
    # pre-chunk into SBUF tile layout: [8 quarters, 128, FC, 512]
    hTq = np.ascontiguousarray(
        hT.reshape(FC, 128, 8, 512).transpose(2, 1, 0, 3))

    def w_pre(W, r0):
        # [H, HDC] feature-major -> [128, FC, HDC]
        wT = np.asarray(W, np.float32)[r0 : r0 + HDC, :].T.astype(bf16)
        return np.ascontiguousarray(wT.reshape(FC, 128, HDC).transpose(1, 0, 2))

    in_maps = []
    for c in range(N_CORES):
        r0 = c * HDC
        woT = np.asarray(Wo, np.float32)[:, r0 : r0 + HDC].T.astype(bf16)
        in_maps.append({
            "hTq": hTq,
            "wqT": w_pre(Wq, r0),
            "wkT": w_pre(Wk, r0),
            "wvT": w_pre(Wv, r0),
            "woT": np.ascontiguousarray(
                woT.reshape(HPC, 128, H).transpose(1, 0, 2)),
            "bq": np.asarray(bq, np.float32)[r0 : r0 + HDC].copy(),
            "bk": np.asarray(bk, np.float32)[r0 : r0 + HDC].copy(),
            "bv": np.asarray(bv, np.float32)[r0 : r0 + HDC].reshape(1, HDC).copy(),
        })

    if "nc" not in _CACHE:
        _CACHE["nc"] = build_program()
    nc = _CACHE["nc"]
    _CACHE["in_maps"] = in_maps

    res = run_bass_kernel_spmd(nc, in_maps, core_ids=list(range(N_CORES)))
    total = np.zeros((T, H), np.float32)
    for r in res.results:
        total += np.asarray(r["out"]).astype(np.float32)
    total += np.asarray(bo, np.float32)[None, :]
    return total.reshape(B, S, H)
